# revision 1
# baseline (speedup 1.0000x reference)
"""Bass/Trainium2 kernel for per-head attention (B=2, S=2048, H=12, DM=768, DH=64).

Sharding: 24 (batch, head) pairs -> 8 cores x 3 pairs. Host pre-transposes the
per-pair activations to [DM, S] (and casts to fp16) so the device reads
contiguous [128, 2048] tiles with d_model on partitions (matmul contraction
dim). The device computes attn_out^T [DM, S] fp32 per pair; the host
transposes back.

Matmul operands are fp16 (full PE rate); PSUM accumulation is fp32.

Math per pair:
  Q^T = W_Q^T X_q^T  [64, S] (6 accumulated d_model chunks; biases are
  structurally zero here -- a bias path auto-enables if they are not)
  K^T, V^T likewise; V natural via 16 PE transposes + a ones column
  -> Vaug [128, 65] per sk tile.
  scores^T block = (K^T block).T @ Q^T group  [128, 512]
  P_u = exp(0.125 * scores^T)  (no max subtraction: |scores| < ~3), masked to
  exact 0 above the diagonal via 4 precomputed 0/1 masks; fully masked
  blocks are skipped.
  Zaug = sum_sk Vaug.T @ P_u  [65, 512]: rows 0:64 unnormalized Z^T, row 64
  = softmax denominators.
  o = (WOaug block).T @ Zaug, WOaug = [W_O[h]; b_O/H]; the PSUM->SBUF
  eviction multiplies by broadcast(1/denom) = attn_out^T exactly.

Scheduling: the PE runs at 2.4 GHz only while its activity stays high (the
HAM clock gate throttles to 1.2 GHz otherwise), and the attention inner loop
alone equilibrates at the ACT exp cadence. So emission interleaves, at
matmul granularity, pair p's attention with pair p+1's projections (which
have no exp dependency), keeps z-matmuls one scores-matmul behind their
exp/mask chain, and carries output projections until their reciprocal
chains are long done.
"""

import numpy as np

B, S, H, DM, DH = 2, 2048, 12, 768, 64
P = 128
NCORES = 8
PPC = (B * H) // NCORES   # pairs per core = 3
NCH = DM // P             # 6 d_model chunks
NG = 4                    # sq groups
GW = S // NG              # 512
NSK = S // P              # 16 sk tiles
VW = DH + 1               # 65 (V augmented with ones column)

NP_IN = np.float16

_NC_CACHE = {}


def _build_bass(use_bias):
    import concourse.mybir as mybir
    import concourse.tile as tile
    from concourse import bacc
    from contextlib import ExitStack

    dt = mybir.dt
    f32 = dt.float32
    f16 = dt.float16
    AF = mybir.ActivationFunctionType

    nc = bacc.Bacc("TRN2", target_bir_lowering=False, debug=False)

    xq = nc.dram_tensor("xqT", [PPC, NCH, P, S], f16, kind="ExternalInput").ap()
    xk = nc.dram_tensor("xkT", [PPC, NCH, P, S], f16, kind="ExternalInput").ap()
    xv = nc.dram_tensor("xvT", [PPC, NCH, P, S], f16, kind="ExternalInput").ap()
    wq = nc.dram_tensor("wq", [PPC, NCH, P, DH], f16, kind="ExternalInput").ap()
    wk = nc.dram_tensor("wk", [PPC, NCH, P, DH], f16, kind="ExternalInput").ap()
    wv = nc.dram_tensor("wv", [PPC, NCH, P, DH], f16, kind="ExternalInput").ap()
    if use_bias:
        bq = nc.dram_tensor("bq", [PPC, 1, DH], f16, kind="ExternalInput").ap()
        bk = nc.dram_tensor("bk", [PPC, 1, DH], f16, kind="ExternalInput").ap()
        bv = nc.dram_tensor("bv", [PPC, 1, DH], f16, kind="ExternalInput").ap()
        onesr = nc.dram_tensor(
            "ones_row", [1, GW], f16, kind="ExternalInput").ap()
    wo = nc.dram_tensor("wo", [PPC, VW, DM], f16, kind="ExternalInput").ap()
    mk = nc.dram_tensor("masks", [NG, P, GW], f16, kind="ExternalInput").ap()
    onesc = nc.dram_tensor("ones_col", [P, NSK, 1], f16, kind="ExternalInput").ap()
    idin = nc.dram_tensor("ident64", [DH, DH], f16, kind="ExternalInput").ap()
    outT = nc.dram_tensor("outT", [PPC, S, DM], f16, kind="ExternalOutput").ap()

    with tile.TileContext(nc) as tc, ExitStack() as ctx:
        consts = ctx.enter_context(tc.tile_pool(name="consts", bufs=1))
        wpool = ctx.enter_context(tc.tile_pool(name="wpool", bufs=2))
        xin = ctx.enter_context(tc.tile_pool(name="xin", bufs=10))
        prj = ctx.enter_context(tc.tile_pool(name="prj", bufs=2))
        expp = ctx.enter_context(tc.tile_pool(name="expp", bufs=6))
        smal = ctx.enter_context(tc.tile_pool(name="smal", bufs=4))
        obuf = ctx.enter_context(tc.tile_pool(name="obuf", bufs=2))
        ps_prj = ctx.enter_context(tc.tile_pool(name="ps_prj", bufs=1, space="PSUM"))
        ps_s2 = ctx.enter_context(tc.tile_pool(name="ps_s2", bufs=2, space="PSUM"))
        ps_att = ctx.enter_context(tc.tile_pool(name="ps_att", bufs=2, space="PSUM"))

        ident = consts.tile([DH, DH], f16)
        nc.sync.dma_start(ident[:], idin)
        masks = consts.tile([P, NG * GW], f16)
        nc.sync.dma_start(
            masks[:].rearrange("p (j c) -> p j c", j=NG),
            mk.rearrange("j p c -> p j c"),
        )
        if use_bias:
            ones = consts.tile([1, GW], f16)
            nc.sync.dma_start(ones[:], onesr)

        # outproj work queue, carried across groups and pairs so the PE
        # only reaches each outproj long after its recip chain completed
        pending = []

        NT = GW // P      # sq tiles per group = 4
        MH = DM // 2      # outproj m-half = 384

        def flush_outproj():
            zaug_, recipT_, p_, g_, wo_sb_ = pending.pop(0)
            ob = obuf.tile([P, NT * DM], f16, tag="ob")
            for t in range(NT):
                for mh in range(2):
                    ms = slice(mh * MH, (mh + 1) * MH)
                    o_ps = ps_att.tile([P, MH], f32, tag="att")
                    nc.tensor.matmul(
                        o_ps[:],
                        lhsT=zaug_[:, t * P:(t + 1) * P],
                        rhs=wo_sb_[:, ms],
                        start=True,
                        stop=True,
                    )
                    nc.vector.tensor_scalar_mul(
                        ob[:, t * DM + mh * MH:t * DM + (mh + 1) * MH],
                        o_ps[:],
                        recipT_[:, t:t + 1],
                    )
                    yield
            nc.sync.dma_start(
                outT[p_, g_ * GW:(g_ + 1) * GW, :].rearrange(
                    "(t q) m -> q t m", q=P),
                ob[:].rearrange("q (t m) -> q t m", t=NT),
            )

        def gen_proj(p, out):
            """Projections + V transposes for pair p; fills out dict."""
            wq_sb = wpool.tile([P, NCH * DH], f16, tag="wq")
            nc.sync.dma_start(
                wq_sb[:].rearrange("p (c e) -> p c e", c=NCH),
                wq[p].rearrange("c p e -> p c e"),
            )
            wk_sb = wpool.tile([P, NCH * DH], f16, tag="wk")
            nc.sync.dma_start(
                wk_sb[:].rearrange("p (c e) -> p c e", c=NCH),
                wk[p].rearrange("c p e -> p c e"),
            )
            wv_sb = wpool.tile([P, NCH * DH], f16, tag="wv")
            nc.sync.dma_start(
                wv_sb[:].rearrange("p (c e) -> p c e", c=NCH),
                wv[p].rearrange("c p e -> p c e"),
            )
            if use_bias:
                bq_sb = wpool.tile([1, DH], f16, tag="bq")
                nc.sync.dma_start(bq_sb[:], bq[p])
                bk_sb = wpool.tile([1, DH], f16, tag="bk")
                nc.sync.dma_start(bk_sb[:], bk[p])
                bv_sb = wpool.tile([1, DH], f16, tag="bv")
                nc.sync.dma_start(bv_sb[:], bv[p])
            wo_sb = wpool.tile([VW, DM], f16, tag="wo")
            nc.sync.dma_start(wo_sb[:], wo[p])
            out["wo"] = wo_sb

            def project(xdram, w_sb, b_sb, tag, dup=False):
                # all 6 chunks stay resident; accumulate the two S-halves
                # sequentially so the projection PSUM is only 2 banks
                xcs = []
                for c in range(NCH):
                    x_c = xin.tile([P, S], f16, tag="xin")
                    nc.sync.dma_start(x_c[:], xdram[p, c])
                    xcs.append(x_c)
                pt_sb = prj.tile([2 * DH if dup else DH, S], f16, tag=tag)
                for half in range(2):
                    pt_ps = ps_prj.tile([DH, S // 2], f32, tag="prj")
                    for c in range(NCH):
                        for gh in range(NG // 2):
                            lo = half * (S // 2) + gh * GW
                            nc.tensor.matmul(
                                pt_ps[:, gh * GW:(gh + 1) * GW],
                                lhsT=w_sb[:, c * DH:(c + 1) * DH],
                                rhs=xcs[c][:, lo:lo + GW],
                                start=(c == 0),
                                stop=(c == NCH - 1) and not use_bias,
                            )
                            yield
                    if use_bias:
                        for gh in range(NG // 2):
                            nc.tensor.matmul(
                                pt_ps[:, gh * GW:(gh + 1) * GW],
                                lhsT=b_sb[:], rhs=ones[:],
                                start=False, stop=True,
                            )
                            yield
                    # split eviction across ACT and DVE; Q^T/K^T are
                    # duplicated onto partitions 64:128 so scores matmuls can
                    # row-pack two K=64 blocks concurrently in the PE array
                    hs = half * (S // 2)
                    nc.scalar.copy(
                        pt_sb[0:DH, hs:hs + S // 4], pt_ps[:, 0:S // 4])
                    nc.vector.tensor_copy(
                        pt_sb[0:DH, hs + S // 4:hs + S // 2],
                        pt_ps[:, S // 4:S // 2])
                    if dup:
                        nc.scalar.copy(
                            pt_sb[DH:2 * DH, hs:hs + S // 4],
                            pt_ps[:, 0:S // 4])
                        nc.vector.tensor_copy(
                            pt_sb[DH:2 * DH, hs + S // 4:hs + S // 2],
                            pt_ps[:, S // 4:S // 2])
                out[tag] = pt_sb

            yield from project(xv, wv_sb, bv_sb if use_bias else None, "vt")
            vt_sb = out["vt"]
            vaug = prj.tile([P, NSK * VW], f16, tag="vaug")
            nc.sync.dma_start(
                vaug[:].rearrange("p (i w) -> p i w", w=VW)[:, :, DH:VW], onesc
            )
            for i in range(NSK):
                tp_ps = ps_att.tile([P, DH], f16, tag="att")
                nc.tensor.transpose(
                    tp_ps[:], vt_sb[:, i * P:(i + 1) * P], ident[:]
                )
                nc.vector.tensor_copy(vaug[:, i * VW:i * VW + DH], tp_ps[:])
                yield
            out["vaug"] = vaug
            yield from project(xq, wq_sb, bq_sb if use_bias else None, "qt", dup=True)
            yield from project(xk, wk_sb, bk_sb if use_bias else None, "kt", dup=True)

        def gen_att(p, tiles):
            qt_sb, kt_sb, vaug, wo_sb = (
                tiles["qt"], tiles["kt"], tiles["vaug"], tiles["wo"])
            for g in range(NG):
                gs = slice(g * GW, (g + 1) * GW)
                nsk = 4 * (g + 1)
                z_ps = ps_att.tile([VW, GW], f32, tag="att")

                def emit_scores_pair(ip):
                    # two sk blocks into one fp16 psum bank -> one exp and
                    # (when on the diagonal) one mask multiply per pair,
                    # halving the ACT per-instruction overhead that
                    # otherwise paces the whole attention loop
                    s_ps = ps_s2.tile([P, 2 * GW], f32, tag="s2")
                    for k in range(2):
                        pb = k * DH
                        nc.tensor.matmul(
                            s_ps[:, k * GW:(k + 1) * GW],
                            lhsT=kt_sb[pb:pb + DH,
                                       (ip + k) * P:(ip + k + 1) * P],
                            rhs=qt_sb[pb:pb + DH, gs],
                            start=True,
                            stop=True,
                            tile_position=(pb, 0),
                        )
                    e_sb = expp.tile([P, 2 * GW], f16, tag="exp")
                    nc.scalar.activation(e_sb[:], s_ps[:], AF.Exp, scale=0.125)
                    if ip >= 4 * g:
                        j = ip - 4 * g
                        em_sb = expp.tile([P, 2 * GW], f16, tag="exp")
                        nc.vector.tensor_mul(
                            em_sb[:], e_sb[:], masks[:, j * GW:(j + 2) * GW]
                        )
                        return em_sb
                    return e_sb

                def emit_z(ip, e_use):
                    for k in range(2):
                        i = ip + k
                        nc.tensor.matmul(
                            z_ps[:],
                            lhsT=vaug[:, i * VW:(i + 1) * VW],
                            rhs=e_use[:, k * GW:(k + 1) * GW],
                            start=(i == 0),
                            stop=(i == nsk - 1),
                        )
                        yield

                # z(ip) emitted after scores(ip+2): the in-order PE queue
                # never parks on pair ip's exp/mask chain
                e_prev = emit_scores_pair(0)
                yield
                for ip in range(2, nsk, 2):
                    e_cur = emit_scores_pair(ip)
                    yield
                    yield from emit_z(ip - 2, e_prev)
                    e_prev = e_cur
                emit_z_final = emit_z(nsk - 2, e_prev)
                yield from emit_z_final

                # evict unnormalized Zaug; transpose the denominator row
                # to [128, 4] so the reciprocal is per-partition (fast) and
                # feeds tensor_scalar in the natural-orientation outproj
                zaug = smal.tile([VW, GW], f16, tag="zaug")
                nc.scalar.copy(zaug[:], z_ps[:])
                sums0 = smal.tile([1, GW], f16, tag="sums0")
                nc.vector.tensor_copy(sums0[:], z_ps[DH:VW, :])
                stp_ps = ps_att.tile([P, 2 * (GW // P)], f16, tag="att")
                for t in range(GW // P):
                    nc.tensor.transpose(
                        stp_ps[:, 2 * t:2 * t + 1],
                        sums0[:, t * P:(t + 1) * P],
                        ident[0:1, 0:1],
                    )
                recipT = smal.tile([P, GW // P], f32, tag="recipT")
                nc.vector.reciprocal(
                    recipT[:], stp_ps[:].rearrange("p (t two) -> p t two", two=2)[:, :, 0])
                pending.append((zaug, recipT, p, g, wo_sb))
                if len(pending) > 2:
                    yield from flush_outproj()

        def interleave(ga, gb):
            while True:
                a_live = b_live = False
                if ga is not None:
                    try:
                        next(ga)
                        a_live = True
                    except StopIteration:
                        ga = None
                if gb is not None:
                    try:
                        next(gb)
                        b_live = True
                    except StopIteration:
                        gb = None
                if not (a_live or b_live):
                    return

        tiles = [{} for _ in range(PPC)]
        interleave(gen_proj(0, tiles[0]), None)
        for p in range(PPC):
            g_next = gen_proj(p + 1, tiles[p + 1]) if p + 1 < PPC else None
            interleave(gen_att(p, tiles[p]), g_next)
        while pending:
            for _ in flush_outproj():
                pass

    nc.compile()
    return nc


def get_nc(use_bias=False):
    if use_bias not in _NC_CACHE:
        _NC_CACHE[use_bias] = _build_bass(use_bias)
    return _NC_CACHE[use_bias]


def _pairs_for_core(c):
    return [(idx // H, idx % H) for idx in range(c * PPC, (c + 1) * PPC)]


def make_masks():
    # mask[j, p, f] = 1.0 iff key pos 128*j + p <= query pos f (within block)
    j = np.arange(NG)[:, None, None]
    p = np.arange(P)[None, :, None]
    f = np.arange(GW)[None, None, :]
    return (f >= P * j + p).astype(NP_IN)


def make_in_maps(inputs, use_bias):
    xq = np.asarray(inputs["normalized_resid_pre_q"], dtype=np.float32)
    xk = np.asarray(inputs["normalized_resid_pre_k"], dtype=np.float32)
    xv = np.asarray(inputs["normalized_resid_pre_v"], dtype=np.float32)
    W_Q = np.asarray(inputs["W_Q"], dtype=np.float32)
    W_K = np.asarray(inputs["W_K"], dtype=np.float32)
    W_V = np.asarray(inputs["W_V"], dtype=np.float32)
    b_Q = np.asarray(inputs["b_Q"], dtype=np.float32)
    b_K = np.asarray(inputs["b_K"], dtype=np.float32)
    b_V = np.asarray(inputs["b_V"], dtype=np.float32)
    W_O = np.asarray(inputs["W_O"], dtype=np.float32)
    b_O = np.asarray(inputs["b_O"], dtype=np.float32)

    masks = make_masks()
    onesc = np.ones((P, NSK, 1), NP_IN)
    ident64 = np.eye(DH, dtype=NP_IN)
    in_maps = []
    for c in range(NCORES):
        pairs = _pairs_for_core(c)
        m = {
            "xqT": np.stack(
                [xq[b, :, h, :].T.astype(NP_IN).reshape(NCH, P, S)
                 for b, h in pairs]),
            "xkT": np.stack(
                [xk[b, :, h, :].T.astype(NP_IN).reshape(NCH, P, S)
                 for b, h in pairs]),
            "xvT": np.stack(
                [xv[b, :, h, :].T.astype(NP_IN).reshape(NCH, P, S)
                 for b, h in pairs]),
            "wq": np.stack(
                [W_Q[h].astype(NP_IN).reshape(NCH, P, DH) for b, h in pairs]),
            "wk": np.stack(
                [W_K[h].astype(NP_IN).reshape(NCH, P, DH) for b, h in pairs]),
            "wv": np.stack(
                [W_V[h].astype(NP_IN).reshape(NCH, P, DH) for b, h in pairs]),
            "wo": np.stack(
                [np.concatenate([W_O[h], (b_O / H)[None, :]], axis=0).astype(NP_IN)
                 for b, h in pairs]),
            "masks": masks,
            "ones_col": onesc,
            "ident64": ident64,
        }
        if use_bias:
            m["bq"] = np.stack([b_Q[h][None, :].astype(NP_IN) for b, h in pairs])
            m["bk"] = np.stack([b_K[h][None, :].astype(NP_IN) for b, h in pairs])
            m["bv"] = np.stack([b_V[h][None, :].astype(NP_IN) for b, h in pairs])
            m["ones_row"] = np.ones((1, GW), NP_IN)
        in_maps.append(m)
    return in_maps


def needs_bias(inputs):
    return any(
        np.any(np.asarray(inputs[k])) for k in ("b_Q", "b_K", "b_V")
    )


def assemble_output(results):
    out = np.empty((B, S, H, DM), np.float32)
    for c in range(NCORES):
        for j, (b, h) in enumerate(_pairs_for_core(c)):
            out[b, :, h, :] = results[c]["outT"][j].astype(np.float32)
    return out


def kernel(**inputs):
    from concourse import bass_utils

    use_bias = needs_bias(inputs)
    nc = get_nc(use_bias)
    in_maps = make_in_maps(inputs, use_bias)
    res = bass_utils.run_bass_kernel_spmd(nc, in_maps, core_ids=list(range(NCORES)))
    return assemble_output(res.results)



# revision 7
# speedup vs baseline: 1.0641x; 1.0641x over previous
"""Bass/Trainium2 kernel for per-head attention (B=2, S=2048, H=12, DM=768, DH=64).

Sharding: 24 (batch, head) pairs -> 8 cores x 3 pairs. Host pre-transposes the
per-pair activations to [DM, S]; xq/xk are sent as fp8e3 (e3m4) and xv as fp16
(V-path quantization propagates ~1:1 to the output; the QK path is dampened by
softmax). Weights are fp16; matmuls mix fp16 lhsT with fp8 rhs (full rate).

Per pair:
  Q^T and K^T are computed COL-PACKED: for each S-quarter, W_Q runs in PE
  columns 0:63 and W_K in columns 64:127 concurrently (tile_position), each
  accumulating 6 d_model chunks into its own PSUM bank (two concurrently-open
  accumulation groups in ONE bank fault the exec unit -- probed). V^T packs the
  same way across S-quarter pairs. Projection matmul slots thus halve.
  Q^T is evicted once and duplicated to partitions 64:128 on GPSIMD; K^T is
  evicted split by sk-block parity (even blocks -> partitions 0:64, odd ->
  64:128), so scores row-packing needs no K duplication.
  scores^T block pair = two K=64 matmuls row-packed at PE rows 0/64.
  P_u = exp(0.125 * scores^T) on ACT; diagonal blocks masked via GPSIMD
  multiply with 0/1 masks. Z runs TWO score-pairs behind (lag-2) so the
  in-order PE queue never parks on an exp/mask chain.
  Zaug = sum_sk Vaug.T @ P_u with a ones column giving softmax denominators in
  row 64. Denominator transposes to [128,4] are deferred into the next group's
  scores stream (their sums-row copy would otherwise head-block the PE queue).
  Outproj o = (WOaug block).T @ Zaug evicted with *1/denom (DVE tensor_scalar
  or ACT activation-scale, balanced), carried in a pending queue until the
  reciprocal chain is long done.

Scheduling: pair 0 emits DMAs + QK projection serially, then attention(p)
interleaves 1:1 with a background stream = [V+transposes of pair p, full
projection of pair p+1], so attention starts ~8us earlier and the PE stays
dense for the HAM clock gate. Subtile deps let attention(0) consume vaug
blocks while later V transposes are still streaming.
"""

import numpy as np
import ml_dtypes

B, S, H, DM, DH = 2, 2048, 12, 768, 64
P = 128
NCORES = 8
PPC = (B * H) // NCORES   # pairs per core = 3
NCH = DM // P             # 6 d_model chunks
NG = 4                    # sq groups
GW = S // NG              # 512
NSK = S // P              # 16 sk tiles
VW = DH + 1               # 65 (V augmented with ones column)
NQ = 4                    # S quarters (= NG)
MH = DM // 2              # outproj m-half = 384
NT = GW // P              # q tiles per group = 4

NP_W = np.float16
NP_X8 = ml_dtypes.float8_e3m4

_NC_CACHE = {}


def _build_bass(use_bias):
    import concourse.mybir as mybir
    import concourse.tile as tile
    from concourse import bacc
    from contextlib import ExitStack

    dt = mybir.dt
    f32 = dt.float32
    f16 = dt.float16
    f8 = dt.float8e3
    AF = mybir.ActivationFunctionType

    nc = bacc.Bacc("TRN2", target_bir_lowering=False, debug=False)

    xq = nc.dram_tensor("xqT", [PPC, NCH, P, S], f8, kind="ExternalInput").ap()
    xk = nc.dram_tensor("xkT", [PPC, NCH, P, S], f8, kind="ExternalInput").ap()
    xv = nc.dram_tensor("xvT", [PPC, NCH, P, S], f16, kind="ExternalInput").ap()
    wq = nc.dram_tensor("wq", [PPC, NCH, P, DH], f16, kind="ExternalInput").ap()
    wk = nc.dram_tensor("wk", [PPC, NCH, P, DH], f16, kind="ExternalInput").ap()
    wv = nc.dram_tensor("wv", [PPC, NCH, P, DH], f16, kind="ExternalInput").ap()
    if use_bias:
        bq = nc.dram_tensor("bq", [PPC, 1, DH], f16, kind="ExternalInput").ap()
        bk = nc.dram_tensor("bk", [PPC, 1, DH], f16, kind="ExternalInput").ap()
        bv = nc.dram_tensor("bv", [PPC, 1, DH], f16, kind="ExternalInput").ap()
        onesr = nc.dram_tensor(
            "ones_row", [1, GW], f16, kind="ExternalInput").ap()
    wo = nc.dram_tensor("wo", [PPC, VW, DM], f16, kind="ExternalInput").ap()
    mk = nc.dram_tensor("masks", [NG, P, GW], f16, kind="ExternalInput").ap()
    onesc = nc.dram_tensor("ones_col", [P, NSK, 1], f16, kind="ExternalInput").ap()
    idin = nc.dram_tensor("ident64", [DH, DH], f16, kind="ExternalInput").ap()
    outT = nc.dram_tensor("outT", [PPC, S, DM], f16, kind="ExternalOutput").ap()

    with tile.TileContext(nc) as tc, ExitStack() as ctx:
        consts = ctx.enter_context(tc.tile_pool(name="consts", bufs=1))
        wpool = ctx.enter_context(tc.tile_pool(name="wpool", bufs=2))
        xin8 = ctx.enter_context(tc.tile_pool(name="xin8", bufs=2))
        xin16 = ctx.enter_context(tc.tile_pool(name="xin16", bufs=2))
        prj = ctx.enter_context(tc.tile_pool(name="prj", bufs=2))
        expp = ctx.enter_context(tc.tile_pool(name="expp", bufs=8))
        smal = ctx.enter_context(tc.tile_pool(name="smal", bufs=4))
        obuf = ctx.enter_context(tc.tile_pool(name="obuf", bufs=2))
        psA = ctx.enter_context(tc.tile_pool(name="psA", bufs=1, space="PSUM"))
        psB = ctx.enter_context(tc.tile_pool(name="psB", bufs=1, space="PSUM"))
        ps_s2 = ctx.enter_context(tc.tile_pool(name="ps_s2", bufs=2, space="PSUM"))
        ps_z = ctx.enter_context(tc.tile_pool(name="ps_z", bufs=1, space="PSUM"))
        ps_o = ctx.enter_context(tc.tile_pool(name="ps_o", bufs=1, space="PSUM"))

        ident = consts.tile([DH, DH], f16)
        nc.sync.dma_start(ident[:], idin)
        masks = consts.tile([P, NG * GW], f16)
        nc.sync.dma_start(
            masks[:].rearrange("p (j c) -> p j c", j=NG),
            mk.rearrange("j p c -> p j c"),
        )
        if use_bias:
            ones = consts.tile([1, GW], f16)
            nc.sync.dma_start(ones[:], onesr)

        # outproj work queue: [zaug, recipT (filled late), p, g, wo_sb]
        pending = []

        def flush_outproj():
            zaug_, recipT_, p_, g_, wo_sb_ = pending.pop(0)
            assert recipT_[0] is not None
            rT = recipT_[0]
            ob = obuf.tile([P, NT * DM], f16, tag="ob")
            for t in range(NT):
                for mh in range(2):
                    o_ps = ps_o.tile([P, MH], f32, tag="o")
                    nc.tensor.matmul(
                        o_ps[:],
                        lhsT=zaug_[:, t * P:(t + 1) * P],
                        rhs=wo_sb_[:, mh * MH:(mh + 1) * MH],
                        start=True,
                        stop=True,
                    )
                    dst = ob[:, t * DM + mh * MH:t * DM + (mh + 1) * MH]
                    if (2 * t + mh) % 4 == 3:
                        nc.scalar.activation(
                            dst, o_ps[:], AF.Copy, scale=rT[:, t:t + 1])
                    else:
                        nc.vector.tensor_scalar_mul(dst, o_ps[:], rT[:, t:t + 1])
                    yield
            nc.sync.dma_start(
                outT[p_, g_ * GW:(g_ + 1) * GW, :].rearrange(
                    "(t q) m -> q t m", q=P),
                ob[:].rearrange("q (t m) -> q t m", t=NT),
            )

        def gen_proj(p, out):
            """DMAs + projections for pair p. Yields "dma" once after DMA
            emission (prime point), "head" after QK is done; V + transposes
            follow as the tail."""
            wq_sb = wpool.tile([P, NCH * DH], f16, tag="wq")
            nc.sync.dma_start(
                wq_sb[:].rearrange("p (c e) -> p c e", c=NCH),
                wq[p].rearrange("c p e -> p c e"),
            )
            wk_sb = wpool.tile([P, NCH * DH], f16, tag="wk")
            nc.sync.dma_start(
                wk_sb[:].rearrange("p (c e) -> p c e", c=NCH),
                wk[p].rearrange("c p e -> p c e"),
            )
            wv_sb = wpool.tile([P, NCH * DH], f16, tag="wv")
            nc.sync.dma_start(
                wv_sb[:].rearrange("p (c e) -> p c e", c=NCH),
                wv[p].rearrange("c p e -> p c e"),
            )
            wo_sb = wpool.tile([VW, DM], f16, tag="wo")
            nc.sync.dma_start(wo_sb[:], wo[p])
            if use_bias:
                bq_sb = wpool.tile([1, DH], f16, tag="bq")
                nc.sync.dma_start(bq_sb[:], bq[p])
                bk_sb = wpool.tile([1, DH], f16, tag="bk")
                nc.sync.dma_start(bk_sb[:], bk[p])
                bv_sb = wpool.tile([1, DH], f16, tag="bv")
                nc.sync.dma_start(bv_sb[:], bv[p])
            out["wo"] = wo_sb

            xq_sb = xin8.tile([P, NCH * S], f8, tag="xq")
            nc.sync.dma_start(
                xq_sb[:].rearrange("p (c s) -> p c s", c=NCH),
                xq[p].rearrange("c p s -> p c s"),
            )
            xk_sb = xin8.tile([P, NCH * S], f8, tag="xk")
            nc.sync.dma_start(
                xk_sb[:].rearrange("p (c s) -> p c s", c=NCH),
                xk[p].rearrange("c p s -> p c s"),
            )
            # xv in two halves so the first V matmul starts sooner
            xv_sb = xin16.tile([P, NCH * S], f16, tag="xv")
            hc = NCH // 2
            for h in range(2):
                nc.sync.dma_start(
                    xv_sb[:, h * hc * S:(h + 1) * hc * S].rearrange(
                        "p (c s) -> p c s", c=hc),
                    xv[p, h * hc:(h + 1) * hc].rearrange("c p s -> p c s"),
                )
            vaug = prj.tile([P, NSK * VW], f16, tag="vaug")
            nc.sync.dma_start(
                vaug[:].rearrange("p (i w) -> p i w", w=VW)[:, :, DH:VW], onesc
            )
            out["vaug"] = vaug
            out["vt_done"] = 0
            yield "dma"

            # ---- QK projection, col-packed per S-quarter ----
            qt = prj.tile([P, S], f16, tag="qt")
            kt = prj.tile([P, S], f16, tag="kt")
            for q in range(NQ):
                pa = psA.tile([P, GW], f32, tag="u")
                pb = psB.tile([P, GW], f32, tag="u")
                qs = slice(q * GW, (q + 1) * GW)
                for c in range(NCH):
                    nc.tensor.matmul(
                        pa[0:DH, :],
                        lhsT=wq_sb[:, c * DH:(c + 1) * DH],
                        rhs=xq_sb[:, c * S + q * GW:c * S + (q + 1) * GW],
                        start=(c == 0),
                        stop=(c == NCH - 1) and not use_bias,
                        tile_position=(0, 0),
                    )
                    nc.tensor.matmul(
                        pb[DH:P, :],
                        lhsT=wk_sb[:, c * DH:(c + 1) * DH],
                        rhs=xk_sb[:, c * S + q * GW:c * S + (q + 1) * GW],
                        start=(c == 0),
                        stop=(c == NCH - 1) and not use_bias,
                        tile_position=(0, 64),
                    )
                    yield
                if use_bias:
                    nc.tensor.matmul(
                        pa[0:DH, :], lhsT=bq_sb[:], rhs=ones[:],
                        start=False, stop=True, tile_position=(0, 0))
                    nc.tensor.matmul(
                        pb[DH:P, :], lhsT=bk_sb[:], rhs=ones[:],
                        start=False, stop=True, tile_position=(0, 64))
                    yield
                # Q: evict lo, duplicate hi on GPSIMD (SBUF->SBUF)
                nc.vector.tensor_copy(qt[0:DH, qs], pa[0:DH, :])
                nc.gpsimd.tensor_copy(qt[DH:P, qs], qt[0:DH, qs])
                # K: parity split -- even sk blocks to rows 0:64, odd to 64:128
                src = pb[DH:P, :].rearrange(
                    "p (b two c) -> p b two c", b=2, two=2)
                de = kt[0:DH, qs].rearrange(
                    "p (b two c) -> p b two c", b=2, two=2)
                do = kt[DH:P, qs].rearrange(
                    "p (b two c) -> p b two c", b=2, two=2)
                nc.scalar.copy(de[:, :, 0, :], src[:, :, 0, :])
                nc.vector.tensor_copy(do[:, :, 1, :], src[:, :, 1, :])
                yield
            out["qt"] = qt
            out["kt"] = kt
            yield "head"

            # ---- V projection, col-packed across S-quarter pairs ----
            vt = prj.tile([DH, S], f16, tag="vt")

            def v_quarter_pair(qp):
                qa, qb = qp, qp + 2
                pa = psA.tile([P, GW], f32, tag="u")
                pb = psB.tile([P, GW], f32, tag="u")
                for c in range(NCH):
                    nc.tensor.matmul(
                        pa[0:DH, :],
                        lhsT=wv_sb[:, c * DH:(c + 1) * DH],
                        rhs=xv_sb[:, c * S + qa * GW:c * S + (qa + 1) * GW],
                        start=(c == 0),
                        stop=(c == NCH - 1) and not use_bias,
                        tile_position=(0, 0),
                    )
                    nc.tensor.matmul(
                        pb[DH:P, :],
                        lhsT=wv_sb[:, c * DH:(c + 1) * DH],
                        rhs=xv_sb[:, c * S + qb * GW:c * S + (qb + 1) * GW],
                        start=(c == 0),
                        stop=(c == NCH - 1) and not use_bias,
                        tile_position=(0, 64),
                    )
                    yield
                if use_bias:
                    nc.tensor.matmul(
                        pa[0:DH, :], lhsT=bv_sb[:], rhs=ones[:],
                        start=False, stop=True, tile_position=(0, 0))
                    nc.tensor.matmul(
                        pb[DH:P, :], lhsT=bv_sb[:], rhs=ones[:],
                        start=False, stop=True, tile_position=(0, 64))
                    yield
                nc.vector.tensor_copy(vt[:, qa * GW:(qa + 1) * GW], pa[0:DH, :])
                nc.scalar.copy(vt[:, qb * GW:(qb + 1) * GW], pb[DH:P, :])
                yield

            def v_transpose(i):
                # alternate psA/psB rings for pipelining
                pool = psA if i % 2 == 0 else psB
                tp = pool.tile([P, DH], f16, tag="u")
                nc.tensor.transpose(
                    tp[:], vt[:, i * P:(i + 1) * P], ident[:]
                )
                nc.vector.tensor_copy(vaug[:, i * VW:i * VW + DH], tp[:])
                out["vt_done"] = i + 1
                yield

            # quarter-pair 0 covers S-quarters 0,2 (sk blocks 0-3, 8-11);
            # transpose blocks 0-3 right after so attention's z (gated on
            # vt_done) can start while the rest of V streams
            yield from v_quarter_pair(0)
            for i in range(4):
                yield from v_transpose(i)
            yield from v_quarter_pair(1)
            for i in range(4, NSK):
                yield from v_transpose(i)

        def gen_att(p, tiles):
            # the background stream emits this pair's projections; spin until
            # the QK tiles exist (each yield advances the background by one)
            while "qt" not in tiles:
                yield
            qt, kt, wo_sb = tiles["qt"], tiles["kt"], tiles["wo"]
            vaug = tiles["vaug"]
            deferred = []

            for g in range(NG):
                gs = slice(g * GW, (g + 1) * GW)
                nsk = 4 * (g + 1)
                zctx = {"ps": None}

                def emit_scores_pair(ip, g=g, gs=gs):
                    s_ps = ps_s2.tile([P, 2 * GW], f32, tag="s2")
                    nc.tensor.matmul(
                        s_ps[:, 0:GW],
                        lhsT=kt[0:DH, ip * P:(ip + 1) * P],
                        rhs=qt[0:DH, gs],
                        start=True, stop=True,
                        tile_position=(0, 0),
                    )
                    nc.tensor.matmul(
                        s_ps[:, GW:2 * GW],
                        lhsT=kt[DH:P, (ip + 1) * P:(ip + 2) * P],
                        rhs=qt[DH:P, gs],
                        start=True, stop=True,
                        tile_position=(64, 0),
                    )
                    e_sb = expp.tile([P, 2 * GW], f16, tag="exp")
                    nc.scalar.activation(e_sb[:], s_ps[:], AF.Exp, scale=0.125)
                    if ip >= 4 * g:
                        j = ip - 4 * g
                        em = expp.tile([P, 2 * GW], f16, tag="exp")
                        nc.gpsimd.tensor_mul(
                            em[:], e_sb[:], masks[:, j * GW:(j + 2) * GW])
                        return em
                    return e_sb

                def emit_z(ip, e_use, nsk=nsk, zctx=zctx):
                    for k in range(2):
                        i = ip + k
                        # emission-order guard: the transpose writing vaug
                        # block i must be EMITTED before this read (the Tile
                        # dep tracker only sees already-emitted writers)
                        while tiles["vt_done"] <= i:
                            yield
                        if zctx["ps"] is None:
                            zctx["ps"] = ps_z.tile(
                                [VW, GW], f32, tag="z", name="z_ps")
                        nc.tensor.matmul(
                            zctx["ps"][:],
                            lhsT=vaug[:, i * VW:(i + 1) * VW],
                            rhs=e_use[:, k * GW:(k + 1) * GW],
                            start=(i == 0),
                            stop=(i == nsk - 1),
                        )
                        yield

                # z runs lag-2 behind scores; deferred stp of the previous
                # group lands after this group's second scores emission
                eq = []
                for ip in range(0, nsk, 2):
                    eq.append((ip, emit_scores_pair(ip)))
                    yield
                    if deferred and len(eq) >= 2:
                        deferred.pop(0)()
                    if len(eq) > 2:
                        ip0, e0 = eq.pop(0)
                        yield from emit_z(ip0, e0)
                while eq:
                    ip0, e0 = eq.pop(0)
                    yield from emit_z(ip0, e0)

                z_ps = zctx["ps"]
                zaug = smal.tile([VW, GW], f16, tag="zaug")
                nc.scalar.copy(zaug[:], z_ps[:])
                sums0 = smal.tile([1, GW], f16, tag="sums0")
                nc.vector.tensor_copy(sums0[:], z_ps[DH:VW, :])
                rslot = [None]
                pending.append([zaug, rslot, p, g, wo_sb])

                def make_stp(sums0=sums0, rslot=rslot):
                    def do():
                        stp = psA.tile([P, 2 * NT], f16, tag="u")
                        for t in range(NT):
                            nc.tensor.transpose(
                                stp[:, 2 * t:2 * t + 1],
                                sums0[:, t * P:(t + 1) * P],
                                ident[0:1, 0:1],
                            )
                        recipT = smal.tile([P, NT], f32, tag="recipT")
                        nc.vector.reciprocal(
                            recipT[:],
                            stp[:].rearrange(
                                "p (t two) -> p t two", two=2)[:, :, 0])
                        rslot[0] = recipT
                    return do

                deferred.append(make_stp())
                if len(pending) > 2:
                    yield from flush_outproj()
            while deferred:
                deferred.pop(0)()

        def interleave(a, b):
            """Pull a and b alternately until a exhausts; b is a shared
            background stream that survives across calls."""
            a_live = True
            while a_live:
                try:
                    next(a)
                except StopIteration:
                    a_live = False
                if b is not None:
                    try:
                        next(b)
                    except StopIteration:
                        b = None
            return b

        def chain(*gens):
            for g in gens:
                yield from g

        tiles = [{} for _ in range(PPC)]
        gens = [gen_proj(p, tiles[p]) for p in range(PPC)]
        # prime pair-0 DMAs + QK projection serially
        for v in gens[0]:
            if v == "head":
                break
        # background: tail of proj(0), then proj(1), proj(2) in sequence
        bg = chain(*gens)
        for p in range(PPC):
            bg = interleave(gen_att(p, tiles[p]), bg)
        while bg is not None:
            try:
                next(bg)
            except StopIteration:
                bg = None
        while pending:
            for _ in flush_outproj():
                pass

    nc.compile()
    return nc


def get_nc(use_bias=False):
    if use_bias not in _NC_CACHE:
        _NC_CACHE[use_bias] = _build_bass(use_bias)
    return _NC_CACHE[use_bias]


def _pairs_for_core(c):
    return [(idx // H, idx % H) for idx in range(c * PPC, (c + 1) * PPC)]


def make_masks():
    # mask[j, p, f] = 1.0 iff key pos 128*j + p <= query pos f (within block)
    j = np.arange(NG)[:, None, None]
    p = np.arange(P)[None, :, None]
    f = np.arange(GW)[None, None, :]
    return (f >= P * j + p).astype(NP_W)


def make_in_maps(inputs, use_bias):
    xq = np.asarray(inputs["normalized_resid_pre_q"], dtype=np.float32)
    xk = np.asarray(inputs["normalized_resid_pre_k"], dtype=np.float32)
    xv = np.asarray(inputs["normalized_resid_pre_v"], dtype=np.float32)
    W_Q = np.asarray(inputs["W_Q"], dtype=np.float32)
    W_K = np.asarray(inputs["W_K"], dtype=np.float32)
    W_V = np.asarray(inputs["W_V"], dtype=np.float32)
    b_Q = np.asarray(inputs["b_Q"], dtype=np.float32)
    b_K = np.asarray(inputs["b_K"], dtype=np.float32)
    b_V = np.asarray(inputs["b_V"], dtype=np.float32)
    W_O = np.asarray(inputs["W_O"], dtype=np.float32)
    b_O = np.asarray(inputs["b_O"], dtype=np.float32)

    masks = make_masks()
    onesc = np.ones((P, NSK, 1), NP_W)
    ident64 = np.eye(DH, dtype=NP_W)
    in_maps = []
    for c in range(NCORES):
        pairs = _pairs_for_core(c)
        m = {
            "xqT": np.stack(
                [xq[b, :, h, :].T.astype(NP_X8).reshape(NCH, P, S)
                 for b, h in pairs]),
            "xkT": np.stack(
                [xk[b, :, h, :].T.astype(NP_X8).reshape(NCH, P, S)
                 for b, h in pairs]),
            "xvT": np.stack(
                [xv[b, :, h, :].T.astype(NP_W).reshape(NCH, P, S)
                 for b, h in pairs]),
            "wq": np.stack(
                [W_Q[h].astype(NP_W).reshape(NCH, P, DH) for b, h in pairs]),
            "wk": np.stack(
                [W_K[h].astype(NP_W).reshape(NCH, P, DH) for b, h in pairs]),
            "wv": np.stack(
                [W_V[h].astype(NP_W).reshape(NCH, P, DH) for b, h in pairs]),
            "wo": np.stack(
                [np.concatenate([W_O[h], (b_O / H)[None, :]], axis=0).astype(NP_W)
                 for b, h in pairs]),
            "masks": masks,
            "ones_col": onesc,
            "ident64": ident64,
        }
        if use_bias:
            m["bq"] = np.stack([b_Q[h][None, :].astype(NP_W) for b, h in pairs])
            m["bk"] = np.stack([b_K[h][None, :].astype(NP_W) for b, h in pairs])
            m["bv"] = np.stack([b_V[h][None, :].astype(NP_W) for b, h in pairs])
            m["ones_row"] = np.ones((1, GW), NP_W)
        in_maps.append(m)
    return in_maps


def needs_bias(inputs):
    return any(
        np.any(np.asarray(inputs[k])) for k in ("b_Q", "b_K", "b_V")
    )


def assemble_output(results):
    out = np.empty((B, S, H, DM), np.float32)
    for c in range(NCORES):
        for j, (b, h) in enumerate(_pairs_for_core(c)):
            out[b, :, h, :] = results[c]["outT"][j].astype(np.float32)
    return out


def kernel(**inputs):
    from concourse import bass_utils

    use_bias = needs_bias(inputs)
    nc = get_nc(use_bias)
    in_maps = make_in_maps(inputs, use_bias)
    res = bass_utils.run_bass_kernel_spmd(nc, in_maps, core_ids=list(range(NCORES)))
    return assemble_output(res.results)


# revision 11
# speedup vs baseline: 1.1570x; 1.0873x over previous
"""Bass/Trainium2 kernel for per-head attention (B=2, S=2048, H=12, DM=768, DH=64).

Sharding: 24 (batch, head) pairs -> 8 cores x 3 pairs. Host pre-transposes the
per-pair activations to [DM, S] in partition-major quarter-blocked layout
[P, NQ, NCH, GW] (one contiguous 3KB DMA line per partition per quarter);
xq/xk are fp8e3 (e3m4), xv fp16 (V-path quantization propagates ~1:1 to the
output; the QK path is dampened by softmax). Weights are fp16; matmuls mix
fp16 lhsT with fp8 rhs at full rate.

Per pair:
  Q^T/K^T/V^T computed per S-quarter as serial M=64 matmuls accumulating 6
  d_model chunks in a single PSUM bank (col-packed concurrent tiles only
  co-stream ~25% of the time -- LDWEIGHTS with a shared row group cannot be
  pulled ahead -- and the second bank is better spent on the outproj).
  K^T is evicted split by sk-block parity (even blocks -> partitions 0:64,
  odd -> 64:128) so score row-packing needs no K duplication; Q^T is evicted
  once and duplicated to partitions 64:128 by an SBUF->SBUF DMA (off the
  compute engines). scores^T pair = two K=64 matmuls row-packed at rows 0/64.
  P_u = exp(0.125 scores^T) on ACT; diagonal blocks masked in place on DVE.
  Z runs two score-pairs behind (lag-2) so the in-order PE queue never parks
  on an exp chain; Zaug (ones column -> denominators in row 64) accumulates in
  one bank. Denominator transposes are deferred into the next group's scores
  stream. Outproj accumulates [128, 2, 512] (two banks, bank-aligned mh
  slices) and evicts both halves in ONE op times 1/denom.

Scheduling: pair 0 emits DMAs + K/Q quarter-0 serially, then attention(p)
interleaves 1:1 with a background stream = [rest of pair p's projections,
pair p+1's projections], so attention starts as soon as the first quarter
lands. Z emission is gated on a vt_done counter (the Tile dep tracker only
sees already-emitted writers). Input DMAs issue from the otherwise-idle
GPSIMD queue so their ring-slot waits never block the output DMAs (Sync).
"""

import numpy as np
import ml_dtypes

B, S, H, DM, DH = 2, 2048, 12, 768, 64
P = 128
NCORES = 8
PPC = (B * H) // NCORES   # pairs per core = 3
NCH = DM // P             # 6 d_model chunks
NG = 4                    # sq groups
GW = S // NG              # 512
NSK = S // P              # 16 sk tiles
VW = DH + 1               # 65 (V augmented with ones column)
NQ = 4                    # S quarters (= NG)
MH = DM // 2              # outproj m-half = 384
NT = GW // P              # q tiles per group = 4

NP_W = np.float16
NP_X8 = ml_dtypes.float8_e3m4

_NC_CACHE = {}


def _build_bass(use_bias):
    import concourse.mybir as mybir
    import concourse.tile as tile
    from concourse import bacc
    from contextlib import ExitStack

    dt = mybir.dt
    f32 = dt.float32
    f16 = dt.float16
    f8 = dt.float8e3
    AF = mybir.ActivationFunctionType

    nc = bacc.Bacc("TRN2", target_bir_lowering=False, debug=False)

    # x layouts: [pair][partition][quarter][chunk][col] (3KB DMA lines)
    xq = nc.dram_tensor("xqT", [PPC, P, NQ, NCH, GW], f8, kind="ExternalInput").ap()
    xk = nc.dram_tensor("xkT", [PPC, P, NQ, NCH, GW], f8, kind="ExternalInput").ap()
    xv = nc.dram_tensor("xvT", [PPC, P, NQ, NCH, GW], f16, kind="ExternalInput").ap()
    # weights: [pair][partition][chunk][e] (p-major, single DMA line/partition)
    wq = nc.dram_tensor("wq", [PPC, P, NCH * DH], f16, kind="ExternalInput").ap()
    wk = nc.dram_tensor("wk", [PPC, P, NCH * DH], f16, kind="ExternalInput").ap()
    wv = nc.dram_tensor("wv", [PPC, P, NCH * DH], f16, kind="ExternalInput").ap()
    if use_bias:
        bq = nc.dram_tensor("bq", [PPC, 1, DH], f16, kind="ExternalInput").ap()
        bk = nc.dram_tensor("bk", [PPC, 1, DH], f16, kind="ExternalInput").ap()
        bv = nc.dram_tensor("bv", [PPC, 1, DH], f16, kind="ExternalInput").ap()
        onesr = nc.dram_tensor(
            "ones_row", [1, GW], f16, kind="ExternalInput").ap()
    wo = nc.dram_tensor("wo", [PPC, VW, DM], f16, kind="ExternalInput").ap()
    mk = nc.dram_tensor("masks", [P, NG * GW], f16, kind="ExternalInput").ap()
    onesc = nc.dram_tensor("ones_col", [P, NSK, 1], f16, kind="ExternalInput").ap()
    idin = nc.dram_tensor("ident64", [DH, DH], f16, kind="ExternalInput").ap()
    # out: [pair][group][partition(q within tile)][t*DM+m]
    outT = nc.dram_tensor("outT", [PPC, NG, P, NT * DM], f16,
                          kind="ExternalOutput").ap()

    with tile.TileContext(nc) as tc, ExitStack() as ctx:
        consts = ctx.enter_context(tc.tile_pool(name="consts", bufs=1))
        wpool = ctx.enter_context(tc.tile_pool(name="wpool", bufs=2))
        xin8 = ctx.enter_context(tc.tile_pool(name="xin8", bufs=2))
        xin16 = ctx.enter_context(tc.tile_pool(name="xin16", bufs=2))
        prj = ctx.enter_context(tc.tile_pool(name="prj", bufs=2))
        expp = ctx.enter_context(tc.tile_pool(name="expp", bufs=8))
        smal = ctx.enter_context(tc.tile_pool(name="smal", bufs=4))
        obuf = ctx.enter_context(tc.tile_pool(name="obuf", bufs=2))
        psA = ctx.enter_context(tc.tile_pool(name="psA", bufs=1, space="PSUM"))
        ps_s2 = ctx.enter_context(tc.tile_pool(name="ps_s2", bufs=2, space="PSUM"))
        ps_z = ctx.enter_context(tc.tile_pool(name="ps_z", bufs=1, space="PSUM"))
        ps_o = ctx.enter_context(tc.tile_pool(name="ps_o", bufs=1, space="PSUM"))

        ident = consts.tile([DH, DH], f16)
        nc.sync.dma_start(ident[:], idin)
        masks = consts.tile([P, NG * GW], f16)
        nc.sync.dma_start(masks[:], mk)
        if use_bias:
            ones = consts.tile([1, GW], f16)
            nc.sync.dma_start(ones[:], onesr)

        # outproj work queue: [zaug, recip-slot (filled late), p, g, wo_sb]
        pending = []

        def flush_outproj():
            zaug_, rslot, p_, g_, wo_sb_ = pending.pop(0)
            assert rslot[0] is not None
            rT = rslot[0]
            ob = obuf.tile([P, NT * DM], f16, tag="ob")
            for t in range(NT):
                o_ps = ps_o.tile([P, 2, GW], f32, tag="o")
                for mh in range(2):
                    nc.tensor.matmul(
                        o_ps[:, mh, 0:MH],
                        lhsT=zaug_[:, t * P:(t + 1) * P],
                        rhs=wo_sb_[:, mh * MH:(mh + 1) * MH],
                        start=True,
                        stop=True,
                    )
                    yield
                dst = ob[:, t * DM:(t + 1) * DM].rearrange(
                    "q (two m) -> q two m", two=2)
                if t == NT - 1:
                    nc.scalar.activation(
                        dst, o_ps[:, :, 0:MH], AF.Copy, scale=rT[:, t:t + 1])
                else:
                    nc.vector.tensor_scalar_mul(
                        dst, o_ps[:, :, 0:MH], rT[:, t:t + 1])
                yield
            nc.sync.dma_start(outT[p_, g_], ob[:])

        def gen_proj(p, out):
            """DMAs + projections for pair p. Yields "dma" once after DMA
            emission (prime point), "head" after K/Q quarter 0."""
            wq_sb = wpool.tile([P, NCH * DH], f16, tag="wq")
            nc.gpsimd.dma_start(wq_sb[:], wq[p])
            wk_sb = wpool.tile([P, NCH * DH], f16, tag="wk")
            nc.gpsimd.dma_start(wk_sb[:], wk[p])
            wv_sb = wpool.tile([P, NCH * DH], f16, tag="wv")
            nc.gpsimd.dma_start(wv_sb[:], wv[p])
            wo_sb = wpool.tile([VW, DM], f16, tag="wo")
            nc.gpsimd.dma_start(wo_sb[:], wo[p])
            if use_bias:
                bq_sb = wpool.tile([1, DH], f16, tag="bq")
                nc.gpsimd.dma_start(bq_sb[:], bq[p])
                bk_sb = wpool.tile([1, DH], f16, tag="bk")
                nc.gpsimd.dma_start(bk_sb[:], bk[p])
                bv_sb = wpool.tile([1, DH], f16, tag="bv")
                nc.gpsimd.dma_start(bv_sb[:], bv[p])
            out["wo"] = wo_sb

            # per-quarter DMAs; head quarters (q=0) first
            xk_sb = xin8.tile([P, NQ * NCH * GW], f8, tag="xk")
            xk_v = xk_sb[:].rearrange("p (q c s) -> p q c s", q=NQ, c=NCH)
            xq_sb = xin8.tile([P, NQ * NCH * GW], f8, tag="xq")
            xq_v = xq_sb[:].rearrange("p (q c s) -> p q c s", q=NQ, c=NCH)
            xv_sb = xin16.tile([P, NQ * NCH * GW], f16, tag="xv")
            xv_v = xv_sb[:].rearrange("p (q c s) -> p q c s", q=NQ, c=NCH)
            for q in range(NQ):
                nc.gpsimd.dma_start(xk_v[:, q], xk[p, :, q])
                nc.gpsimd.dma_start(xq_v[:, q], xq[p, :, q])
            for q in range(NQ):
                nc.gpsimd.dma_start(xv_v[:, q], xv[p, :, q])
            vaug = prj.tile([P, NSK * VW], f16, tag="vaug")
            nc.sync.dma_start(
                vaug[:].rearrange("p (i w) -> p i w", w=VW)[:, :, DH:VW], onesc
            )
            out["vaug"] = vaug
            out["vt_done"] = 0
            out["qk_done"] = 0
            qt = prj.tile([P, S], f16, tag="qt")
            kt = prj.tile([P, S], f16, tag="kt")
            yield "dma"

            def project_quarter(w_sb, b_sb, x_v, q):
                ps = psA.tile([DH, GW], f32, tag="u", name="prj_ps")
                for c in range(NCH):
                    nc.tensor.matmul(
                        ps[:],
                        lhsT=w_sb[:, c * DH:(c + 1) * DH],
                        rhs=x_v[:, q, c, :],
                        start=(c == 0),
                        stop=(c == NCH - 1) and not use_bias,
                    )
                    yield
                if use_bias:
                    nc.tensor.matmul(
                        ps[:], lhsT=b_sb[:], rhs=ones[:],
                        start=False, stop=True)
                    yield
                return ps

            def k_quarter(q):
                ps = yield from project_quarter(
                    wk_sb, bk_sb if use_bias else None, xk_v, q)
                qs = slice(q * GW, (q + 1) * GW)
                src = ps[:].rearrange("p (b two c) -> p b two c", b=2, two=2)
                de = kt[0:DH, qs].rearrange(
                    "p (b two c) -> p b two c", b=2, two=2)
                do = kt[DH:P, qs].rearrange(
                    "p (b two c) -> p b two c", b=2, two=2)
                nc.scalar.copy(de[:, :, 0, :], src[:, :, 0, :])
                nc.vector.tensor_copy(do[:, :, 1, :], src[:, :, 1, :])
                yield
                yield

            def q_quarter(q):
                ps = yield from project_quarter(
                    wq_sb, bq_sb if use_bias else None, xq_v, q)
                qs = slice(q * GW, (q + 1) * GW)
                nc.vector.tensor_copy(qt[0:DH, qs], ps[:])
                # duplicate to partitions 64:128 off-engine (SBUF->SBUF DMA)
                nc.gpsimd.dma_start(qt[DH:P, qs], qt[0:DH, qs])
                yield
                yield

            yield from k_quarter(0)
            yield from q_quarter(0)
            out["qt"] = qt
            out["kt"] = kt
            out["qk_done"] = 1
            yield "head"
            for q in range(1, NQ):
                yield from k_quarter(q)
                yield from q_quarter(q)
                out["qk_done"] = q + 1

            # ---- V projection + transposes, quarter by quarter ----
            vt = prj.tile([DH, S], f16, tag="vt")
            for q in range(NQ):
                ps = yield from project_quarter(
                    wv_sb, bv_sb if use_bias else None, xv_v, q)
                nc.scalar.copy(vt[:, q * GW:(q + 1) * GW], ps[:])
                yield
                for i in range(4 * q, 4 * q + 4):
                    tp = psA.tile([P, DH], f16, tag="u", name="vtr_ps")
                    nc.tensor.transpose(
                        tp[:], vt[:, i * P:(i + 1) * P], ident[:]
                    )
                    nc.vector.tensor_copy(vaug[:, i * VW:i * VW + DH], tp[:])
                    out["vt_done"] = i + 1
                    yield

        def gen_att(p, tiles):
            # the background stream emits this pair's projections; spin until
            # the QK tiles exist (each yield advances the background by one)
            while "qt" not in tiles:
                yield
            qt, kt, wo_sb = tiles["qt"], tiles["kt"], tiles["wo"]
            vaug = tiles["vaug"]
            deferred = []

            for g in range(NG):
                # emission-order guard: scores of group g read qt quarter g
                # and kt quarters 0..g; their evictions must be emitted first
                while tiles["qk_done"] <= g:
                    yield
                gs = slice(g * GW, (g + 1) * GW)
                nsk = 4 * (g + 1)
                zctx = {"ps": None}

                def emit_scores_pair(ip, g=g, gs=gs):
                    s_ps = ps_s2.tile([P, 2 * GW], f32, tag="s2")
                    nc.tensor.matmul(
                        s_ps[:, 0:GW],
                        lhsT=kt[0:DH, ip * P:(ip + 1) * P],
                        rhs=qt[0:DH, gs],
                        start=True, stop=True,
                        tile_position=(0, 0),
                    )
                    nc.tensor.matmul(
                        s_ps[:, GW:2 * GW],
                        lhsT=kt[DH:P, (ip + 1) * P:(ip + 2) * P],
                        rhs=qt[DH:P, gs],
                        start=True, stop=True,
                        tile_position=(64, 0),
                    )
                    e_sb = expp.tile([P, 2 * GW], f16, tag="exp")
                    nc.scalar.activation(e_sb[:], s_ps[:], AF.Exp, scale=0.125)
                    if ip >= 4 * g:
                        j = ip - 4 * g
                        nc.vector.tensor_mul(
                            e_sb[:], e_sb[:], masks[:, j * GW:(j + 2) * GW])
                    return e_sb

                def emit_z(ip, e_use, nsk=nsk, zctx=zctx):
                    for k in range(2):
                        i = ip + k
                        # emission-order guard: the transpose writing vaug
                        # block i must be EMITTED before this read (the Tile
                        # dep tracker only sees already-emitted writers)
                        while tiles["vt_done"] <= i:
                            yield
                        if zctx["ps"] is None:
                            zctx["ps"] = ps_z.tile(
                                [VW, GW], f32, tag="z", name="z_ps")
                        nc.tensor.matmul(
                            zctx["ps"][:],
                            lhsT=vaug[:, i * VW:(i + 1) * VW],
                            rhs=e_use[:, k * GW:(k + 1) * GW],
                            start=(i == 0),
                            stop=(i == nsk - 1),
                        )
                        yield

                # z runs lag-2 behind scores; deferred stp of the previous
                # group lands after this group's second scores emission
                eq = []
                for ip in range(0, nsk, 2):
                    eq.append((ip, emit_scores_pair(ip)))
                    yield
                    if deferred and len(eq) >= 2:
                        deferred.pop(0)()
                    if len(eq) > 2:
                        ip0, e0 = eq.pop(0)
                        yield from emit_z(ip0, e0)
                while eq:
                    ip0, e0 = eq.pop(0)
                    yield from emit_z(ip0, e0)

                z_ps = zctx["ps"]
                zaug = smal.tile([VW, GW], f16, tag="zaug")
                nc.scalar.copy(zaug[:], z_ps[:])
                sums0 = smal.tile([1, GW], f16, tag="sums0")
                nc.vector.tensor_copy(sums0[:], z_ps[DH:VW, :])
                rslot = [None]
                pending.append([zaug, rslot, p, g, wo_sb])

                def make_stp(sums0=sums0, rslot=rslot):
                    def do():
                        stp = psA.tile([P, 2 * NT], f16, tag="u", name="stp_ps")
                        for t in range(NT):
                            nc.tensor.transpose(
                                stp[:, 2 * t:2 * t + 1],
                                sums0[:, t * P:(t + 1) * P],
                                ident[0:1, 0:1],
                            )
                        recipT = smal.tile([P, NT], f32, tag="recipT")
                        nc.vector.reciprocal(
                            recipT[:],
                            stp[:].rearrange(
                                "p (t two) -> p t two", two=2)[:, :, 0])
                        rslot[0] = recipT
                    return do

                deferred.append(make_stp())
                if len(pending) > 2:
                    yield from flush_outproj()
            while deferred:
                deferred.pop(0)()

        def interleave(a, b):
            """Pull a and b alternately until a exhausts; b is a shared
            background stream that survives across calls."""
            a_live = True
            while a_live:
                try:
                    next(a)
                except StopIteration:
                    a_live = False
                if b is not None:
                    try:
                        next(b)
                    except StopIteration:
                        b = None
            return b

        def chain(*gens):
            for g in gens:
                yield from g

        tiles = [{} for _ in range(PPC)]
        gens = [gen_proj(p, tiles[p]) for p in range(PPC)]
        # prime pair-0 DMAs + first K/Q quarter serially
        for v in gens[0]:
            if v == "head":
                break
        # background: rest of proj(0), then proj(1), proj(2)
        bg = chain(*gens)
        for p in range(PPC):
            bg = interleave(gen_att(p, tiles[p]), bg)
        while bg is not None:
            try:
                next(bg)
            except StopIteration:
                bg = None
        while pending:
            for _ in flush_outproj():
                pass

    nc.compile()
    return nc


def get_nc(use_bias=False):
    if use_bias not in _NC_CACHE:
        _NC_CACHE[use_bias] = _build_bass(use_bias)
    return _NC_CACHE[use_bias]


def _pairs_for_core(c):
    return [(idx // H, idx % H) for idx in range(c * PPC, (c + 1) * PPC)]


def make_masks():
    # mask[p, (j c)] = 1.0 iff key pos 128*j + p <= query pos c (within block)
    j = np.arange(NG)[None, :, None]
    p = np.arange(P)[:, None, None]
    f = np.arange(GW)[None, None, :]
    return (f >= P * j + p).astype(NP_W).reshape(P, NG * GW)


def _xT_quarters(x, b, h, np_dt):
    # [S, DM] -> [DM, S] -> [P, NQ, NCH, GW] (partition-major quarter blocks)
    xt = x[b, :, h, :].T.astype(np_dt)          # [DM, S]
    xt = xt.reshape(NCH, P, NQ, GW)
    return np.ascontiguousarray(xt.transpose(1, 2, 0, 3))


def make_in_maps(inputs, use_bias):
    xq = np.asarray(inputs["normalized_resid_pre_q"], dtype=np.float32)
    xk = np.asarray(inputs["normalized_resid_pre_k"], dtype=np.float32)
    xv = np.asarray(inputs["normalized_resid_pre_v"], dtype=np.float32)
    W_Q = np.asarray(inputs["W_Q"], dtype=np.float32)
    W_K = np.asarray(inputs["W_K"], dtype=np.float32)
    W_V = np.asarray(inputs["W_V"], dtype=np.float32)
    b_Q = np.asarray(inputs["b_Q"], dtype=np.float32)
    b_K = np.asarray(inputs["b_K"], dtype=np.float32)
    b_V = np.asarray(inputs["b_V"], dtype=np.float32)
    W_O = np.asarray(inputs["W_O"], dtype=np.float32)
    b_O = np.asarray(inputs["b_O"], dtype=np.float32)

    def w_pmajor(W):
        # [DM, DH] -> [NCH, P, DH] -> [P, NCH*DH]
        w = W.astype(NP_W).reshape(NCH, P, DH)
        return np.ascontiguousarray(w.transpose(1, 0, 2)).reshape(P, NCH * DH)

    masks = make_masks()
    onesc = np.ones((P, NSK, 1), NP_W)
    ident64 = np.eye(DH, dtype=NP_W)
    in_maps = []
    for c in range(NCORES):
        pairs = _pairs_for_core(c)
        m = {
            "xqT": np.stack([_xT_quarters(xq, b, h, NP_X8) for b, h in pairs]),
            "xkT": np.stack([_xT_quarters(xk, b, h, NP_X8) for b, h in pairs]),
            "xvT": np.stack([_xT_quarters(xv, b, h, NP_W) for b, h in pairs]),
            "wq": np.stack([w_pmajor(W_Q[h]) for b, h in pairs]),
            "wk": np.stack([w_pmajor(W_K[h]) for b, h in pairs]),
            "wv": np.stack([w_pmajor(W_V[h]) for b, h in pairs]),
            "wo": np.stack(
                [np.concatenate([W_O[h], (b_O / H)[None, :]], axis=0).astype(NP_W)
                 for b, h in pairs]),
            "masks": masks,
            "ones_col": onesc,
            "ident64": ident64,
        }
        if use_bias:
            m["bq"] = np.stack([b_Q[h][None, :].astype(NP_W) for b, h in pairs])
            m["bk"] = np.stack([b_K[h][None, :].astype(NP_W) for b, h in pairs])
            m["bv"] = np.stack([b_V[h][None, :].astype(NP_W) for b, h in pairs])
            m["ones_row"] = np.ones((1, GW), NP_W)
        in_maps.append(m)
    return in_maps


def needs_bias(inputs):
    return any(
        np.any(np.asarray(inputs[k])) for k in ("b_Q", "b_K", "b_V")
    )


def assemble_output(results):
    out = np.empty((B, S, H, DM), np.float32)
    for c in range(NCORES):
        for j, (b, h) in enumerate(_pairs_for_core(c)):
            # outT[j]: [NG, P, NT*DM] with row q = query g*GW + t*P + q
            o = results[c]["outT"][j].astype(np.float32)
            o = o.reshape(NG, P, NT, DM).transpose(0, 2, 1, 3).reshape(S, DM)
            out[b, :, h, :] = o
    return out


def kernel(**inputs):
    from concourse import bass_utils

    use_bias = needs_bias(inputs)
    nc = get_nc(use_bias)
    in_maps = make_in_maps(inputs, use_bias)
    res = bass_utils.run_bass_kernel_spmd(nc, in_maps, core_ids=list(range(NCORES)))
    return assemble_output(res.results)


# revision 20
# speedup vs baseline: 1.2245x; 1.0583x over previous
"""Bass/Trainium2 kernel for per-head attention (B=2, S=2048, H=12, DM=768, DH=64).

Sharding: 24 (batch, head) pairs -> 8 cores x 3 pairs. Host pre-transposes the
per-pair activations to [DM, S] in partition-major quarter-blocked layout
[P, NQ, NCH, GW] (one contiguous 3KB DMA line per partition per quarter);
xq/xk are fp8e3 (e3m4), xv fp16 (V-path quantization propagates ~1:1 to the
output; the QK path is dampened by softmax). Weights are fp16; matmuls mix
fp16 lhsT with fp8 rhs at full rate.

Per pair:
  Q^T/K^T/V^T computed per S-quarter as serial M=64 matmuls accumulating 6
  d_model chunks in a single PSUM bank (col-packed concurrent tiles only
  co-stream ~25% of the time -- LDWEIGHTS with a shared row group cannot be
  pulled ahead -- and the second bank is better spent on the outproj).
  K^T is evicted split by sk-block parity (even blocks -> partitions 0:64,
  odd -> 64:128) so score row-packing needs no K duplication; Q^T is evicted
  once and duplicated to partitions 64:128 by an SBUF->SBUF DMA (off the
  compute engines). scores^T pair = two K=64 matmuls row-packed at rows 0/64.
  P_u = exp(0.125 scores^T) on ACT; diagonal blocks masked in place on DVE.
  Z runs two score-pairs behind (lag-2) so the in-order PE queue never parks
  on an exp chain; Zaug (ones column -> denominators in row 64) accumulates in
  one bank. Denominator transposes are deferred into the next group's scores
  stream. Outproj accumulates [128, 2, 512] (two banks, bank-aligned mh
  slices) and evicts both halves in ONE op times 1/denom.

Scheduling: pair 0 emits DMAs + K/Q quarter-0 serially, then attention(p)
interleaves 1:1 with a background stream = [rest of pair p's projections,
pair p+1's projections], so attention starts as soon as the first quarter
lands. Z emission is gated on a vt_done counter (the Tile dep tracker only
sees already-emitted writers). Input DMAs issue from the otherwise-idle
GPSIMD queue so their ring-slot waits never block the output DMAs (Sync).
"""

import numpy as np
import ml_dtypes

B, S, H, DM, DH = 2, 2048, 12, 768, 64
P = 128
NCORES = 8
PPC = (B * H) // NCORES   # pairs per core = 3
NCH = DM // P             # 6 d_model chunks
NG = 4                    # sq groups
GW = S // NG              # 512
NSK = S // P              # 16 sk tiles
VW = DH + 1               # 65 (V augmented with ones column)
NQ = 4                    # S quarters (= NG)
MH = DM // 2              # outproj m-half = 384
NT = GW // P              # q tiles per group = 4

NP_W = np.float16
NP_X8 = ml_dtypes.float8_e3m4

_NC_CACHE = {}


def _build_bass(use_bias):
    import concourse.mybir as mybir
    import concourse.tile as tile
    from concourse import bacc
    from contextlib import ExitStack

    dt = mybir.dt
    f32 = dt.float32
    f16 = dt.float16
    f8 = dt.float8e3
    AF = mybir.ActivationFunctionType

    nc = bacc.Bacc("TRN2", target_bir_lowering=False, debug=False)

    # x layouts: [pair][partition][quarter][chunk][col] (3KB DMA lines)
    xq = nc.dram_tensor("xqT", [PPC, P, NQ, NCH, GW], f8, kind="ExternalInput").ap()
    xk = nc.dram_tensor("xkT", [PPC, P, NQ, NCH, GW], f8, kind="ExternalInput").ap()
    xv = nc.dram_tensor("xvT", [PPC, P, NQ, NCH, GW], f16, kind="ExternalInput").ap()
    # weights: [pair][partition][chunk][e] (p-major, single DMA line/partition)
    wq = nc.dram_tensor("wq", [PPC, P, NCH * DH], f16, kind="ExternalInput").ap()
    wk = nc.dram_tensor("wk", [PPC, P, NCH * DH], f16, kind="ExternalInput").ap()
    wv = nc.dram_tensor("wv", [PPC, P, NCH * DH], f16, kind="ExternalInput").ap()
    if use_bias:
        bq = nc.dram_tensor("bq", [PPC, 1, DH], f16, kind="ExternalInput").ap()
        bk = nc.dram_tensor("bk", [PPC, 1, DH], f16, kind="ExternalInput").ap()
        bv = nc.dram_tensor("bv", [PPC, 1, DH], f16, kind="ExternalInput").ap()
        onesr = nc.dram_tensor(
            "ones_row", [1, GW], f16, kind="ExternalInput").ap()
    wo = nc.dram_tensor("wo", [PPC, VW, DM], f16, kind="ExternalInput").ap()
    mk = nc.dram_tensor("masks", [P, NG * GW], f16, kind="ExternalInput").ap()
    onesc = nc.dram_tensor("ones_col", [P, NSK, 1], f16, kind="ExternalInput").ap()
    idin = nc.dram_tensor("ident64", [DH, DH], f16, kind="ExternalInput").ap()
    # out: [pair][group][partition(q within tile)][t*DM+m]
    outT = nc.dram_tensor("outT", [PPC, NG, P, NT * DM], f16,
                          kind="ExternalOutput").ap()

    with tile.TileContext(nc) as tc, ExitStack() as ctx:
        consts = ctx.enter_context(tc.tile_pool(name="consts", bufs=1))
        wpool = ctx.enter_context(tc.tile_pool(name="wpool", bufs=2))
        xin8 = ctx.enter_context(tc.tile_pool(name="xin8", bufs=2))
        xin16 = ctx.enter_context(tc.tile_pool(name="xin16", bufs=2))
        prj = ctx.enter_context(tc.tile_pool(name="prj", bufs=2))
        expp = ctx.enter_context(tc.tile_pool(name="expp", bufs=8))
        smal = ctx.enter_context(tc.tile_pool(name="smal", bufs=4))
        obuf = ctx.enter_context(tc.tile_pool(name="obuf", bufs=2))
        psA = ctx.enter_context(tc.tile_pool(name="psA", bufs=1, space="PSUM"))
        ps_s2 = ctx.enter_context(tc.tile_pool(name="ps_s2", bufs=2, space="PSUM"))
        ps_z = ctx.enter_context(tc.tile_pool(name="ps_z", bufs=1, space="PSUM"))
        ps_o = ctx.enter_context(tc.tile_pool(name="ps_o", bufs=1, space="PSUM"))

        ident = consts.tile([P, DH], f16)
        nc.sync.dma_start(ident[0:DH, :], idin)
        nc.sync.dma_start(ident[DH:P, :], idin)
        masks = consts.tile([P, NG * GW], f16)
        nc.sync.dma_start(masks[:], mk)
        if use_bias:
            ones = consts.tile([1, GW], f16)
            nc.sync.dma_start(ones[:], onesr)

        # outproj work queue: [zaug, recip-slot (filled late), p, g, wo_sb]
        pending = []

        def flush_outproj(drain=False):
            zaug_, rslot, p_, g_, wo_sb_ = pending.pop(0)
            assert rslot[0] is not None
            rT = rslot[0]
            ob = obuf.tile([P, NT * DM], f16, tag="ob")
            for t in range(NT):
                if drain and t % 2 == 1:
                    # at drain time the scores banks are free; alternate into
                    # them so mm(t+1) does not wait on evict(t)
                    o_ps = ps_s2.tile([P, 2, GW], f32, tag="s2", name="o_ps")
                else:
                    o_ps = ps_o.tile([P, 2, GW], f32, tag="o", name="o_ps")
                for mh in range(2):
                    nc.tensor.matmul(
                        o_ps[:, mh, 0:MH],
                        lhsT=zaug_[:, t * P:(t + 1) * P],
                        rhs=wo_sb_[:, mh * MH:(mh + 1) * MH],
                        start=True,
                        stop=True,
                    )
                    yield
                dst = ob[:, t * DM:(t + 1) * DM].rearrange(
                    "q (two m) -> q two m", two=2)
                if t == NT - 1:
                    nc.scalar.activation(
                        dst, o_ps[:, :, 0:MH], AF.Copy, scale=rT[:, t:t + 1])
                else:
                    nc.vector.tensor_scalar_mul(
                        dst, o_ps[:, :, 0:MH], rT[:, t:t + 1])
                yield
                yield
            nc.sync.dma_start(outT[p_, g_], ob[:])

        def gen_proj(p, out):
            """DMAs + projections for pair p. Yields "dma" once after DMA
            emission (prime point), "head" after K/Q quarter 0."""
            # head-critical DMAs first: wk + xk quarter 0, wq + xq quarter 0
            wk_sb = wpool.tile([P, NCH * DH], f16, tag="wk")
            nc.gpsimd.dma_start(wk_sb[:], wk[p])
            xk_sb = xin8.tile([P, NQ * NCH * GW], f8, tag="xk")
            xk_v = xk_sb[:].rearrange("p (q c s) -> p q c s", q=NQ, c=NCH)
            nc.gpsimd.dma_start(xk_v[:, 0], xk[p, :, 0])
            wq_sb = wpool.tile([P, NCH * DH], f16, tag="wq")
            nc.gpsimd.dma_start(wq_sb[:], wq[p])
            xq_sb = xin8.tile([P, NQ * NCH * GW], f8, tag="xq")
            xq_v = xq_sb[:].rearrange("p (q c s) -> p q c s", q=NQ, c=NCH)
            nc.gpsimd.dma_start(xq_v[:, 0], xq[p, :, 0])
            wv_sb = wpool.tile([P, NCH * DH], f16, tag="wv")
            nc.gpsimd.dma_start(wv_sb[:], wv[p])
            wo_sb = wpool.tile([VW, DM], f16, tag="wo")
            nc.gpsimd.dma_start(wo_sb[:], wo[p])
            if use_bias:
                bq_sb = wpool.tile([1, DH], f16, tag="bq")
                nc.gpsimd.dma_start(bq_sb[:], bq[p])
                bk_sb = wpool.tile([1, DH], f16, tag="bk")
                nc.gpsimd.dma_start(bk_sb[:], bk[p])
                bv_sb = wpool.tile([1, DH], f16, tag="bv")
                nc.gpsimd.dma_start(bv_sb[:], bv[p])
            out["wo"] = wo_sb

            xv_sb = xin16.tile([P, NQ * NCH * GW], f16, tag="xv")
            xv_v = xv_sb[:].rearrange("p (q c s) -> p q c s", q=NQ, c=NCH)
            for q in range(1, NQ):
                nc.gpsimd.dma_start(xk_v[:, q], xk[p, :, q])
                nc.gpsimd.dma_start(xq_v[:, q], xq[p, :, q])
            for q in range(NQ):
                nc.gpsimd.dma_start(xv_v[:, q], xv[p, :, q])
            vaug = prj.tile([P, NSK * VW], f16, tag="vaug")
            nc.sync.dma_start(
                vaug[:].rearrange("p (i w) -> p i w", w=VW)[:, :, DH:VW], onesc
            )
            out["vaug"] = vaug
            out["vt_done"] = 0
            out["qk_done"] = 0
            qt = prj.tile([P, S], f16, tag="qt")
            kt = prj.tile([P, S], f16, tag="kt")
            yield "dma"

            def project_quarter(w_sb, b_sb, x_v, q):
                ps = psA.tile([DH, GW], f32, tag="u", name="prj_ps")
                for c in range(NCH):
                    nc.tensor.matmul(
                        ps[:],
                        lhsT=w_sb[:, c * DH:(c + 1) * DH],
                        rhs=x_v[:, q, c, :],
                        start=(c == 0),
                        stop=(c == NCH - 1) and not use_bias,
                    )
                    yield
                if use_bias:
                    nc.tensor.matmul(
                        ps[:], lhsT=b_sb[:], rhs=ones[:],
                        start=False, stop=True)
                    yield
                return ps

            def k_quarter(q):
                ps = yield from project_quarter(
                    wk_sb, bk_sb if use_bias else None, xk_v, q)
                qs = slice(q * GW, (q + 1) * GW)
                src = ps[:].rearrange("p (b two c) -> p b two c", b=2, two=2)
                de = kt[0:DH, qs].rearrange(
                    "p (b two c) -> p b two c", b=2, two=2)
                do = kt[DH:P, qs].rearrange(
                    "p (b two c) -> p b two c", b=2, two=2)
                nc.scalar.copy(de[:, :, 0, :], src[:, :, 0, :])
                nc.vector.tensor_copy(do[:, :, 1, :], src[:, :, 1, :])
                yield
                yield

            def q_quarter(q):
                ps = yield from project_quarter(
                    wq_sb, bq_sb if use_bias else None, xq_v, q)
                qs = slice(q * GW, (q + 1) * GW)
                nc.vector.tensor_copy(qt[0:DH, qs], ps[:])
                # duplicate to partitions 64:128 off-engine (SBUF->SBUF DMA)
                nc.gpsimd.dma_start(qt[DH:P, qs], qt[0:DH, qs])
                yield
                yield

            yield from k_quarter(0)
            yield from q_quarter(0)
            out["qt"] = qt
            out["kt"] = kt
            out["qk_done"] = 1
            yield "head"
            for q in range(1, NQ):
                yield from k_quarter(q)
                yield from q_quarter(q)
                out["qk_done"] = q + 1

            # ---- V projection + transposes, quarter by quarter ----
            vt = prj.tile([DH, S], f16, tag="vt")
            for q in range(NQ):
                ps = yield from project_quarter(
                    wv_sb, bv_sb if use_bias else None, xv_v, q)
                nc.scalar.copy(vt[:, q * GW:(q + 1) * GW], ps[:])
                yield
                for i in range(4 * q, 4 * q + 4):
                    tp = psA.tile([P, DH], f16, tag="u", name="vtr_ps")
                    nc.tensor.transpose(
                        tp[:], vt[:, i * P:(i + 1) * P], ident[0:DH, :]
                    )
                    nc.vector.tensor_copy(vaug[:, i * VW:i * VW + DH], tp[:])
                    out["vt_done"] = i + 1
                    yield

        def gen_att(p, tiles):
            # the background stream emits this pair's projections; spin until
            # the QK tiles exist (each yield advances the background by one)
            while "qt" not in tiles:
                yield
            qt, kt, wo_sb = tiles["qt"], tiles["kt"], tiles["wo"]
            vaug = tiles["vaug"]
            deferred = []

            for g in range(NG):
                # emission-order guard: scores of group g read qt quarter g
                # and kt quarters 0..g; their evictions must be emitted first
                while tiles["qk_done"] <= g:
                    yield
                gs = slice(g * GW, (g + 1) * GW)
                nsk = 4 * (g + 1)
                zctx = {"ps": None}

                def emit_scores_pair(ip, g=g, gs=gs):
                    s_ps = ps_s2.tile([P, 2 * GW], f32, tag="s2")
                    nc.tensor.matmul(
                        s_ps[:, 0:GW],
                        lhsT=kt[0:DH, ip * P:(ip + 1) * P],
                        rhs=qt[0:DH, gs],
                        start=True, stop=True,
                        tile_position=(0, 0),
                    )
                    nc.tensor.matmul(
                        s_ps[:, GW:2 * GW],
                        lhsT=kt[DH:P, (ip + 1) * P:(ip + 2) * P],
                        rhs=qt[DH:P, gs],
                        start=True, stop=True,
                        tile_position=(64, 0),
                    )
                    e_sb = expp.tile([P, 2 * GW], f16, tag="exp")
                    nc.scalar.activation(e_sb[:], s_ps[:], AF.Exp, scale=0.125)
                    if ip >= 4 * g:
                        j = ip - 4 * g
                        nc.vector.tensor_mul(
                            e_sb[:], e_sb[:], masks[:, j * GW:(j + 2) * GW])
                    return e_sb

                def emit_z(ip, e_use, nsk=nsk, zctx=zctx):
                    for k in range(2):
                        i = ip + k
                        # emission-order guard: the transpose writing vaug
                        # block i must be EMITTED before this read (the Tile
                        # dep tracker only sees already-emitted writers)
                        while tiles["vt_done"] <= i:
                            yield
                        if zctx["ps"] is None:
                            zctx["ps"] = ps_z.tile(
                                [VW, GW], f32, tag="z", name="z_ps")
                        nc.tensor.matmul(
                            zctx["ps"][:],
                            lhsT=vaug[:, i * VW:(i + 1) * VW],
                            rhs=e_use[:, k * GW:(k + 1) * GW],
                            start=(i == 0),
                            stop=(i == nsk - 1),
                        )
                        yield

                # z runs lag-2 behind scores; deferred stp of the previous
                # group lands after this group's second scores emission
                eq = []
                for ip in range(0, nsk, 2):
                    eq.append((ip, emit_scores_pair(ip)))
                    yield
                    if deferred and len(eq) >= 2:
                        deferred.pop(0)()
                    if len(eq) > 2:
                        ip0, e0 = eq.pop(0)
                        yield from emit_z(ip0, e0)
                while eq:
                    ip0, e0 = eq.pop(0)
                    yield from emit_z(ip0, e0)

                z_ps = zctx["ps"]
                zaug = smal.tile([VW, GW], f16, tag="zaug")
                nc.scalar.copy(zaug[:], z_ps[:])
                rslot = [None]
                pending.append([zaug, rslot, p, g, wo_sb])

                def make_stp(zaug=zaug, rslot=rslot):
                    def do():
                        # transpose the denominator row straight out of zaug
                        # (partition 64 -> PE row tile (64, 0))
                        stp = psA.tile([P, 2 * NT], f16, tag="u", name="stp_ps")
                        for t in range(NT):
                            nc.tensor.transpose(
                                stp[:, 2 * t:2 * t + 1],
                                zaug[DH:VW, t * P:(t + 1) * P],
                                ident[DH:DH + 1, 0:1],
                                tile_position=(64, 0),
                            )
                        recipT = smal.tile([P, NT], f32, tag="recipT")
                        nc.vector.reciprocal(
                            recipT[:],
                            stp[:].rearrange(
                                "p (t two) -> p t two", two=2)[:, :, 0])
                        rslot[0] = recipT
                    return do

                deferred.append(make_stp())
                if len(pending) > 1:
                    yield from flush_outproj()
            while deferred:
                deferred.pop(0)()

        def interleave(a, b):
            """Pull a and b alternately until a exhausts; b is a shared
            background stream that survives across calls."""
            a_live = True
            while a_live:
                try:
                    next(a)
                except StopIteration:
                    a_live = False
                if b is not None:
                    try:
                        next(b)
                    except StopIteration:
                        b = None
            return b

        def chain(*gens):
            for g in gens:
                yield from g

        tiles = [{} for _ in range(PPC)]
        gens = [gen_proj(p, tiles[p]) for p in range(PPC)]
        # prime pair-0 DMAs + first K/Q quarter serially
        for v in gens[0]:
            if v == "head":
                break
        # background: rest of proj(0), then proj(1), proj(2)
        bg = chain(*gens)
        for p in range(PPC):
            bg = interleave(gen_att(p, tiles[p]), bg)
        while bg is not None:
            try:
                next(bg)
            except StopIteration:
                bg = None
        while pending:
            for _ in flush_outproj(drain=True):
                pass

    nc.compile()
    return nc


def get_nc(use_bias=False):
    if use_bias not in _NC_CACHE:
        _NC_CACHE[use_bias] = _build_bass(use_bias)
    return _NC_CACHE[use_bias]


def _pairs_for_core(c):
    return [(idx // H, idx % H) for idx in range(c * PPC, (c + 1) * PPC)]


def make_masks():
    # mask[p, (j c)] = 1.0 iff key pos 128*j + p <= query pos c (within block)
    j = np.arange(NG)[None, :, None]
    p = np.arange(P)[:, None, None]
    f = np.arange(GW)[None, None, :]
    return (f >= P * j + p).astype(NP_W).reshape(P, NG * GW)


def _xT_quarters(x, b, h, np_dt):
    # [S, DM] -> [DM, S] -> [P, NQ, NCH, GW] (partition-major quarter blocks)
    xt = x[b, :, h, :].T.astype(np_dt)          # [DM, S]
    xt = xt.reshape(NCH, P, NQ, GW)
    return np.ascontiguousarray(xt.transpose(1, 2, 0, 3))


def make_in_maps(inputs, use_bias):
    xq = np.asarray(inputs["normalized_resid_pre_q"], dtype=np.float32)
    xk = np.asarray(inputs["normalized_resid_pre_k"], dtype=np.float32)
    xv = np.asarray(inputs["normalized_resid_pre_v"], dtype=np.float32)
    W_Q = np.asarray(inputs["W_Q"], dtype=np.float32)
    W_K = np.asarray(inputs["W_K"], dtype=np.float32)
    W_V = np.asarray(inputs["W_V"], dtype=np.float32)
    b_Q = np.asarray(inputs["b_Q"], dtype=np.float32)
    b_K = np.asarray(inputs["b_K"], dtype=np.float32)
    b_V = np.asarray(inputs["b_V"], dtype=np.float32)
    W_O = np.asarray(inputs["W_O"], dtype=np.float32)
    b_O = np.asarray(inputs["b_O"], dtype=np.float32)

    def w_pmajor(W):
        # [DM, DH] -> [NCH, P, DH] -> [P, NCH*DH]
        w = W.astype(NP_W).reshape(NCH, P, DH)
        return np.ascontiguousarray(w.transpose(1, 0, 2)).reshape(P, NCH * DH)

    masks = make_masks()
    onesc = np.ones((P, NSK, 1), NP_W)
    ident64 = np.eye(DH, dtype=NP_W)
    in_maps = []
    for c in range(NCORES):
        pairs = _pairs_for_core(c)
        m = {
            "xqT": np.stack([_xT_quarters(xq, b, h, NP_X8) for b, h in pairs]),
            "xkT": np.stack([_xT_quarters(xk, b, h, NP_X8) for b, h in pairs]),
            "xvT": np.stack([_xT_quarters(xv, b, h, NP_W) for b, h in pairs]),
            "wq": np.stack([w_pmajor(W_Q[h]) for b, h in pairs]),
            "wk": np.stack([w_pmajor(W_K[h]) for b, h in pairs]),
            "wv": np.stack([w_pmajor(W_V[h]) for b, h in pairs]),
            "wo": np.stack(
                [np.concatenate([W_O[h], (b_O / H)[None, :]], axis=0).astype(NP_W)
                 for b, h in pairs]),
            "masks": masks,
            "ones_col": onesc,
            "ident64": ident64,
        }
        if use_bias:
            m["bq"] = np.stack([b_Q[h][None, :].astype(NP_W) for b, h in pairs])
            m["bk"] = np.stack([b_K[h][None, :].astype(NP_W) for b, h in pairs])
            m["bv"] = np.stack([b_V[h][None, :].astype(NP_W) for b, h in pairs])
            m["ones_row"] = np.ones((1, GW), NP_W)
        in_maps.append(m)
    return in_maps


def needs_bias(inputs):
    return any(
        np.any(np.asarray(inputs[k])) for k in ("b_Q", "b_K", "b_V")
    )


def assemble_output(results):
    out = np.empty((B, S, H, DM), np.float32)
    for c in range(NCORES):
        for j, (b, h) in enumerate(_pairs_for_core(c)):
            # outT[j]: [NG, P, NT*DM] with row q = query g*GW + t*P + q
            o = results[c]["outT"][j].astype(np.float32)
            o = o.reshape(NG, P, NT, DM).transpose(0, 2, 1, 3).reshape(S, DM)
            out[b, :, h, :] = o
    return out


def kernel(**inputs):
    from concourse import bass_utils

    use_bias = needs_bias(inputs)
    nc = get_nc(use_bias)
    in_maps = make_in_maps(inputs, use_bias)
    res = bass_utils.run_bass_kernel_spmd(nc, in_maps, core_ids=list(range(NCORES)))
    return assemble_output(res.results)
